# revision 20
# baseline (speedup 1.0000x reference)
"""Trainium2 Bass kernel for nn_CrossAttentionBlock_73452530696666.

Math note: the reference's attention softmax runs over a single KV token, so
attn == 1.0 exactly and the whole q/scores path is dead code. The output
reduces to, per batch b and spatial position s:

    p[b]   = (text_emb[b] @ Wv.T) @ Wo.T + bo          # (C,) per batch
    y[:,s] = LayerNorm_C(x[:, s] + p[b]) * gamma + beta

p is a tiny per-batch matvec chain -> computed on host. The device kernel is
a fused bias-add + LayerNorm over the channel dim streamed over (B, C, H*W).

v4 design (HW-measured op costs per [128,2048] bf16: DVE TS@4x 746ns,
DVE TT@2x 1217ns, DVE STT 1x 2813ns (!), ACT Square 1819ns, ACT
Square+biasAP 2414ns (!), ACT copy [128,1024] 1146ns, PE f32 K=1 matmul
~860-1460ns vs bf16 ~300ns):
  - baseline value path kept (z=TS(x+p)@4x in-place, t=TT(z,rst16),
    y=TT(t,mb16)) -- STT fusion is 1x on HW, a loss.
  - stats rows (rstd, mb) written BF16 so the K=1 partition-broadcast
    matmuls run at bf16 rate on PE (f32 matmuls are ~4x slower).
  - GPS_YADD value-phase adds offloaded to gpsimd (only spare engine).
  - xp bufs=4: the whole per-core x input stays resident -> input DMA
    never stalls on buffer recycling.
  - optional int8 output (scale folded into rstd/mb), host dequant.

Sharding: data-parallel over batch, 2 batches per core on 8 cores.
"""

import sys

sys.path.insert(0, "/opt/trn_rl_repo")

import ml_dtypes
import numpy as np

B, C, H, W, T = 16, 512, 64, 64, 768
S = H * W  # 4096
NCORES = 8
BPC = B // NCORES  # batches per core = 2
NCH = C // 128  # channel chunks = 4
MACRO = 2048  # spatial columns per macro tile
SUB = 512  # matmul / PSUM sub tile
NSUB = MACRO // SUB  # 4
NMACRO = S // MACRO  # 2 per batch
EPS = 1e-5

# Tuning knobs (rebuild to change):
GPS_YADD = 0  # value-phase yadd TTs per macro on gpsimd (0..4) -- gpsimd
              # shares the DVE SBUF port; >0 measured as a net loss
GPS_SQ = 0    # square TTs per macro on gpsimd (0..4)
CP_DVE = 0    # of the 4 [128,1024] PSUM->SBUF bcast copies per macro, on DVE
OUT_I8 = False  # write y as int8: DVE TT with i8 output drops to 1x (+17us
                # DVE, measured) for -4us of DMA -- net loss, keep bf16

I8_SCALE = 5.0 / 127.0  # y ~ N(0,1) per token; clip at 5 sigma

# Set by test harness to request a profiled run.
TRACE = False
LAST_RESULTS = None

_CACHE = {}


def _build(trivial_affine: bool):
    import concourse.bass as bass
    import concourse.tile as tile
    from concourse import bacc, mybir

    f32 = mybir.dt.float32
    bf16 = mybir.dt.bfloat16
    i8 = mybir.dt.int8
    AF = mybir.ActivationFunctionType
    OP = mybir.AluOpType

    out_dt = i8 if OUT_I8 else bf16
    # int8 scale k folded into rstd (and thus mb = -mu*rstd) via the
    # rsqrt activation's scale: rsqrt(var/k^2) = k*rstd.
    k = 1.0 / I8_SCALE if OUT_I8 else 1.0
    inv_k2 = 1.0 / (k * k)

    nc = bacc.Bacc("TRN2", target_bir_lowering=False)
    x = nc.dram_tensor("x", (BPC, C, S), bf16, kind="ExternalInput")
    pcol = nc.dram_tensor("pcol", (128, NCH, BPC), f32, kind="ExternalInput")
    if not trivial_affine:
        gcols = nc.dram_tensor("gcols", (128, NCH), f32, kind="ExternalInput")
        bcols = nc.dram_tensor("bcols", (128, NCH), f32, kind="ExternalInput")
    y = nc.dram_tensor("y", (BPC, C, S), out_dt, kind="ExternalOutput")

    xv = x.rearrange("b (n p) s -> b p n s", p=128)
    yv = y.rearrange("b (n p) s -> b p n s", p=128)

    with tile.TileContext(nc) as tc:
        with tc.tile_pool(name="consts", bufs=1) as consts:
            ones_c = consts.tile([128, 1], bf16)
            nc.vector.memset(ones_c, 1.0 / C)  # lhsT for channel-mean matmuls
            ones97 = consts.tile([97, 128], bf16)
            nc.vector.memset(ones97, 1.0)  # bf16 lhsT rows for K=1 broadcasts
            pcol_sb = consts.tile([128, NCH, BPC], f32)
            nc.sync.dma_start(pcol_sb, pcol[:, :, :])
            # preload both ACT function tables during the first x DMA
            warm = consts.tile([1, 1], f32)
            nc.scalar.activation(warm, ones_c[0:1, 0:1], AF.Square)
            warm2 = consts.tile([1, 1], f32)
            nc.scalar.activation(warm2, warm, AF.Abs_reciprocal_sqrt)
            if not trivial_affine:
                g_sb = consts.tile([128, NCH], f32)
                nc.sync.dma_start(g_sb, gcols[:, :])
                b_sb = consts.tile([128, NCH], f32)
                nc.sync.dma_start(b_sb, bcols[:, :])

            with (
                tc.tile_pool(name="xp", bufs=4) as xp,
                tc.tile_pool(name="sqp", bufs=2) as sqp,
                tc.tile_pool(name="bcastp", bufs=4) as bcastp,
                tc.tile_pool(name="rowp", bufs=2) as rowp,
                tc.tile_pool(name="t16p", bufs=2) as t16p,
                tc.tile_pool(name="yp", bufs=2) as yp,
                tc.tile_pool(name="mup", bufs=2, space="PSUM") as mup,
                tc.tile_pool(name="e2p", bufs=2, space="PSUM") as e2p,
                tc.tile_pool(name="bcp", bufs=2, space="PSUM") as bcp,
            ):
             def stage1a(b, m, first=False):
                """load + z + sq + PE reductions (no stats chain)"""
                s0 = m * MACRO
                xt = xp.tile([128, NCH, MACRO], bf16, name="xt")
                if first:
                    # quarter the first DMA so compute starts ~5us sooner
                    q = MACRO // 4
                    for qi in range(4):
                        nc.sync.dma_start(
                            xt[:, :, q * qi:q * (qi + 1)],
                            xv[b, :, :, s0 + q * qi:s0 + q * (qi + 1)],
                        )
                else:
                    nc.sync.dma_start(xt, xv[b, :, :, s0:s0 + MACRO])

                # z = x + p (bf16, TS @4x), in place over xt
                zt = xt
                nz = 4 if first else 1
                for ci in range(NCH):
                    for qi in range(nz):
                        w = MACRO // nz
                        nc.vector.tensor_scalar_add(
                            zt[:, ci, w * qi:w * (qi + 1)],
                            xt[:, ci, w * qi:w * (qi + 1)],
                            pcol_sb[:, ci, b:b + 1],
                        )

                # sq = z^2 (bf16): ONE wide ACT Square over all 4 chunks
                sq = sqp.tile([128, NCH, MACRO], bf16, name="sq", tag="sq")
                nc.scalar.activation(sq, zt, AF.Square)

                # channel sums: row j lives at partition 32*j of one bank
                mu_all = mup.tile([97, SUB], f32, name="mu_all")
                e2_all = e2p.tile([97, SUB], f32, name="e2_all")
                for ci in range(NCH):
                    for j in range(NSUB):
                        sl = slice(SUB * j, SUB * (j + 1))
                        mrow = mu_all[32 * j:32 * j + 1, :]
                        nc.tensor.matmul(
                            mrow, ones_c, zt[:, ci, sl],
                            start=(ci == 0), stop=(ci == NCH - 1),
                            tile_position=(0, 32 * j),
                        )
                for ci in range(NCH):
                    for j in range(NSUB):
                        sl = slice(SUB * j, SUB * (j + 1))
                        erow = e2_all[32 * j:32 * j + 1, :]
                        nc.tensor.matmul(
                            erow, ones_c, sq[:, ci, sl],
                            start=(ci == 0), stop=(ci == NCH - 1),
                            tile_position=(0, 32 * j),
                        )
                return b, zt, mu_all, e2_all, s0

             def stage1b(st):
                """stats chain + broadcasts + copies.

                Emitted AFTER the previous macro's tmul block so the DVE
                queue never blocks the value phase on the stats chain.
                """
                b, zt, mu_all, e2_all, s0 = st
                # rstd/mb written BF16 so the K=1 broadcast matmuls run at
                # PE bf16 rate (f32 matmuls are ~4x slower / column).
                musq = rowp.tile([97, SUB], f32, tag="musq")
                nc.scalar.activation(musq, mu_all, AF.Square)
                var = rowp.tile([97, SUB], f32, tag="var")
                nc.vector.scalar_tensor_tensor(
                    var, e2_all, float(EPS), musq, op0=OP.add, op1=OP.subtract,
                )
                rstd = rowp.tile([97, SUB], bf16, tag="rstd")
                # Abs_reciprocal_sqrt = rsqrt(|x|); var >= 0 so identical to
                # rsqrt, and unlike AF.Rsqrt it is not accuracy-gated.
                nc.scalar.activation(
                    rstd, var, AF.Abs_reciprocal_sqrt, scale=inv_k2,
                )
                mb_row = rowp.tile([97, SUB], bf16, tag="mb_row")
                nc.vector.scalar_tensor_tensor(
                    mb_row, mu_all, -1.0, rstd, op0=OP.mult, op1=OP.mult,
                )

                rst16 = bcastp.tile([128, 1, MACRO], bf16, tag="rst16")
                mb16 = bcastp.tile([128, 1, MACRO], bf16, tag="mb16")
                ncopy = 0
                for half in range(2):
                    hsl = slice(1024 * half, 1024 * (half + 1))
                    rp = bcp.tile([128, 1024], f32, name="rp", tag="bc")
                    mp = bcp.tile([128, 1024], f32, name="mp", tag="bc")
                    for jj in range(2):
                        j = 2 * half + jj
                        pr = 32 * j
                        psl = slice(SUB * jj, SUB * (jj + 1))
                        nc.tensor.matmul(
                            rp[:, psl], ones97[pr:pr + 1, :],
                            rstd[pr:pr + 1, :],
                            start=True, stop=True, tile_position=(pr, 0),
                        )
                        nc.tensor.matmul(
                            mp[:, psl], ones97[pr:pr + 1, :],
                            mb_row[pr:pr + 1, :],
                            start=True, stop=True, tile_position=(pr, 0),
                        )
                    for dst, src in ((rst16, rp), (mb16, mp)):
                        if ncopy < CP_DVE:
                            nc.vector.tensor_copy(dst[:, 0, hsl], src)
                        else:
                            nc.scalar.copy(dst[:, 0, hsl], src)
                        ncopy += 1

                return b, zt, rst16, mb16, s0

             def stage2_tmul(vst):
                """value multiply t = z * rst16: ONE wide TT, rst16
                repeated over the 4 chunks via a stride-0 middle dim"""
                b, zt, rst16, mb16, s0 = vst
                t16 = t16p.tile([128, NCH, MACRO], bf16, tag="t16")
                nc.vector.tensor_mul(
                    t16, zt, rst16.to_broadcast((128, NCH, MACRO))
                )
                return t16

             def stage2_yadd(vst, t16):
                """value add y = t + mb16 (one wide TT) + one output DMA"""
                b, zt, rst16, mb16, s0 = vst
                yt = yp.tile([128, NCH, MACRO], out_dt, name="yc", tag="yc")
                nc.vector.tensor_tensor(
                    yt, t16, mb16.to_broadcast((128, NCH, MACRO)), op=OP.add,
                )
                if not trivial_affine:
                    for ci in range(NCH):
                        nc.vector.tensor_scalar(
                            yt[:, ci, :], yt[:, ci, :],
                            g_sb[:, ci:ci + 1], b_sb[:, ci:ci + 1],
                            op0=OP.mult, op1=OP.add,
                        )
                nc.sync.dma_start(yv[b, :, :, s0:s0 + MACRO], yt)

             bms = [(b, m) for b in range(BPC) for m in range(NMACRO)]
             pending = None
             for k, bm in enumerate(bms):
                st = stage1a(*bm, first=(k == 0))
                if pending is not None:
                    t16 = stage2_tmul(pending)
                vst = stage1b(st)
                if pending is not None:
                    stage2_yadd(pending, t16)
                pending = vst
             t16 = stage2_tmul(pending)
             stage2_yadd(pending, t16)

    nc.compile()
    return nc


def _get_module(trivial_affine: bool):
    key = (trivial_affine, GPS_YADD, GPS_SQ, CP_DVE, OUT_I8, MACRO)
    if key not in _CACHE:
        _CACHE[key] = _build(trivial_affine)
    return _CACHE[key]


def kernel(**inputs) -> np.ndarray:
    global LAST_RESULTS
    from concourse.bass_utils import run_bass_kernel_spmd

    x = np.asarray(inputs["x"], dtype=np.float32)
    te = np.asarray(inputs["text_emb"], dtype=np.float32)
    Wv = np.asarray(inputs["Wv"], dtype=np.float32)
    Wo = np.asarray(inputs["Wo"], dtype=np.float32)
    bo = np.asarray(inputs["bo"], dtype=np.float32)
    gamma = np.asarray(inputs["gamma"], dtype=np.float32)
    beta = np.asarray(inputs["beta"], dtype=np.float32)
    assert x.shape == (B, C, H, W), x.shape

    trivial = bool(np.all(gamma == 1.0) and np.all(beta == 0.0))
    nc = _get_module(trivial)

    # host-side tiny matvec chain: p[b] = (te @ Wv.T) @ Wo.T + bo
    p = (te @ Wv.T) @ Wo.T + bo  # (B, C) f32
    pcol = np.ascontiguousarray(
        p.reshape(B, NCH, 128).transpose(2, 1, 0)
    )  # (128, NCH, B)

    x16 = np.ascontiguousarray(
        x.reshape(B, C, S).astype(ml_dtypes.bfloat16)
    )

    in_maps = []
    for c in range(NCORES):
        bsl = slice(BPC * c, BPC * (c + 1))
        m = {
            "x": np.ascontiguousarray(x16[bsl]),
            "pcol": np.ascontiguousarray(pcol[:, :, bsl]),
        }
        if not trivial:
            m["gcols"] = np.ascontiguousarray(gamma.reshape(NCH, 128).T)
            m["bcols"] = np.ascontiguousarray(beta.reshape(NCH, 128).T)
        in_maps.append(m)

    kwargs = {}
    if TRACE:
        import os
        import shutil

        shutil.rmtree("/tmp/bassprof", ignore_errors=True)
        os.makedirs("/tmp/bassprof", exist_ok=True)
        kwargs["tmpdir"] = "/tmp/bassprof"
    res = run_bass_kernel_spmd(
        nc, in_maps, core_ids=list(range(NCORES)), trace=TRACE, **kwargs
    )
    LAST_RESULTS = res
    out = np.concatenate(
        [np.asarray(res.results[c]["y"]) for c in range(NCORES)], axis=0
    )
    out = out.astype(np.float32)
    if OUT_I8:
        out *= I8_SCALE
    return np.ascontiguousarray(out.reshape(B, C, H, W))


# revision 26
# speedup vs baseline: 1.1025x; 1.1025x over previous
"""Trainium2 Bass kernel for nn_CrossAttentionBlock_73452530696666.

Math note: the reference's attention softmax runs over a single KV token, so
attn == 1.0 exactly and the whole q/scores path is dead code. The output
reduces to, per batch b and spatial position s:

    p[b]   = (text_emb[b] @ Wv.T) @ Wo.T + bo          # (C,) per batch
    y[:,s] = LayerNorm_C(x[:, s] + p[b]) * gamma + beta

p is a tiny per-batch matvec chain -> computed on host. The device kernel is
a fused bias-add + LayerNorm over the channel dim streamed over (B, C, H*W).

v4 design (HW-measured op costs per [128,2048] bf16: DVE TS@4x 746ns,
DVE TT@2x 1217ns, DVE STT 1x 2813ns (!), ACT Square 1819ns, ACT
Square+biasAP 2414ns (!), ACT copy [128,1024] 1146ns, PE f32 K=1 matmul
~860-1460ns vs bf16 ~300ns):
  - baseline value path kept (z=TS(x+p)@4x in-place, t=TT(z,rst16),
    y=TT(t,mb16)) -- STT fusion is 1x on HW, a loss.
  - stats rows (rstd, mb) written BF16 so the K=1 partition-broadcast
    matmuls run at bf16 rate on PE (f32 matmuls are ~4x slower).
  - GPS_YADD value-phase adds offloaded to gpsimd (only spare engine).
  - xp bufs=4: the whole per-core x input stays resident -> input DMA
    never stalls on buffer recycling.
  - optional int8 output (scale folded into rstd/mb), host dequant.

Sharding: data-parallel over batch, 2 batches per core on 8 cores.
"""

import sys

sys.path.insert(0, "/opt/trn_rl_repo")

import ml_dtypes
import numpy as np

B, C, H, W, T = 16, 512, 64, 64, 768
S = H * W  # 4096
NCORES = 8
BPC = B // NCORES  # batches per core = 2
NCH = C // 128  # channel chunks = 4
MACRO = 2048  # spatial columns per macro tile
SUB = 512  # matmul / PSUM sub tile
NSUB = MACRO // SUB  # 4
NMACRO = S // MACRO  # 2 per batch
EPS = 1e-5

# Tuning knobs (rebuild to change):
GPS_YADD = 0  # value-phase yadd TTs per macro on gpsimd (0..4) -- gpsimd
              # shares the DVE SBUF port; >0 measured as a net loss
GPS_SQ = 0    # square TTs per macro on gpsimd (0..4)
CP_DVE = 0    # of the 4 [128,1024] PSUM->SBUF bcast copies per macro, on DVE
OUT_I8 = False  # write y as int8: DVE TT with i8 output drops to 1x (+17us
                # DVE, measured) for -4us of DMA -- net loss, keep bf16

I8_SCALE = 5.0 / 127.0  # y ~ N(0,1) per token; clip at 5 sigma

# Set by test harness to request a profiled run.
TRACE = False
LAST_RESULTS = None

_CACHE = {}


def _build(trivial_affine: bool):
    import concourse.bass as bass
    import concourse.tile as tile
    from concourse import bacc, mybir

    f32 = mybir.dt.float32
    bf16 = mybir.dt.bfloat16
    i8 = mybir.dt.int8
    AF = mybir.ActivationFunctionType
    OP = mybir.AluOpType

    out_dt = i8 if OUT_I8 else bf16
    # int8 scale k folded into rstd (and thus mb = -mu*rstd) via the
    # rsqrt activation's scale: rsqrt(var/k^2) = k*rstd.
    k = 1.0 / I8_SCALE if OUT_I8 else 1.0
    inv_k2 = 1.0 / (k * k)

    nc = bacc.Bacc("TRN2", target_bir_lowering=False)
    x = nc.dram_tensor("x", (BPC, C, S), bf16, kind="ExternalInput")
    pcol = nc.dram_tensor("pcol", (128, NCH, BPC), f32, kind="ExternalInput")
    if not trivial_affine:
        gcols = nc.dram_tensor("gcols", (128, NCH), f32, kind="ExternalInput")
        bcols = nc.dram_tensor("bcols", (128, NCH), f32, kind="ExternalInput")
    y = nc.dram_tensor("y", (BPC, C, S), out_dt, kind="ExternalOutput")

    xv = x.rearrange("b (n p) s -> b p n s", p=128)
    yv = y.rearrange("b (n p) s -> b p n s", p=128)

    with tile.TileContext(nc) as tc:
        with tc.tile_pool(name="consts", bufs=1) as consts:
            ones_c = consts.tile([128, 1], bf16)
            nc.vector.memset(ones_c, 1.0 / C)  # lhsT for channel-mean matmuls
            ones97 = consts.tile([97, 128], bf16)
            nc.vector.memset(ones97, 1.0)  # bf16 lhsT rows for K=1 broadcasts
            pcol_sb = consts.tile([128, NCH, BPC], f32)
            nc.sync.dma_start(pcol_sb, pcol[:, :, :])
            # preload both ACT function tables during the first x DMA
            warm = consts.tile([1, 1], f32)
            nc.scalar.activation(warm, ones_c[0:1, 0:1], AF.Square)
            warm2 = consts.tile([1, 1], f32)
            nc.scalar.activation(warm2, warm, AF.Abs_reciprocal_sqrt)
            if not trivial_affine:
                g_sb = consts.tile([128, NCH], f32)
                nc.sync.dma_start(g_sb, gcols[:, :])
                b_sb = consts.tile([128, NCH], f32)
                nc.sync.dma_start(b_sb, bcols[:, :])

            with (
                tc.tile_pool(name="xp", bufs=4) as xp,
                tc.tile_pool(name="sqp", bufs=6) as sqp,
                tc.tile_pool(name="bcastp", bufs=4) as bcastp,
                tc.tile_pool(name="rowp", bufs=6) as rowp,
                tc.tile_pool(name="t16p", bufs=4) as t16p,
                tc.tile_pool(name="yp", bufs=6) as yp,
                tc.tile_pool(name="mup", bufs=2, space="PSUM") as mup,
                tc.tile_pool(name="e2p", bufs=2, space="PSUM") as e2p,
                tc.tile_pool(name="bcp", bufs=2, space="PSUM") as bcp,
            ):
             def stage1a(b, m, first=False):
                """load + z + sq + PE reductions (no stats chain)"""
                s0 = m * MACRO
                xt = xp.tile([128, NCH, MACRO], bf16, name="xt")
                if first:
                    # quarter the first DMA so compute starts ~5us sooner
                    q = MACRO // 4
                    for qi in range(4):
                        nc.sync.dma_start(
                            xt[:, :, q * qi:q * (qi + 1)],
                            xv[b, :, :, s0 + q * qi:s0 + q * (qi + 1)],
                        )
                else:
                    nc.sync.dma_start(xt, xv[b, :, :, s0:s0 + MACRO])

                # z = x + p (bf16, TS @4x), in place over xt
                zt = xt
                nz = 4 if first else 1
                for ci in range(NCH):
                    for qi in range(nz):
                        w = MACRO // nz
                        nc.vector.tensor_scalar_add(
                            zt[:, ci, w * qi:w * (qi + 1)],
                            xt[:, ci, w * qi:w * (qi + 1)],
                            pcol_sb[:, ci, b:b + 1],
                        )

                # sq = z^2 (bf16) per chunk on ACT
                sq_tiles = []
                for ci in range(NCH):
                    sq = sqp.tile([128, MACRO], bf16, name=f"sq{ci}", tag="sq")
                    nc.scalar.activation(sq, zt[:, ci, :], AF.Square)
                    sq_tiles.append(sq)

                # channel sums: row j lives at partition 32*j of one bank
                mu_all = mup.tile([97, SUB], f32, name="mu_all")
                e2_all = e2p.tile([97, SUB], f32, name="e2_all")
                for ci in range(NCH):
                    for j in range(NSUB):
                        sl = slice(SUB * j, SUB * (j + 1))
                        mrow = mu_all[32 * j:32 * j + 1, :]
                        nc.tensor.matmul(
                            mrow, ones_c, zt[:, ci, sl],
                            start=(ci == 0), stop=(ci == NCH - 1),
                            tile_position=(0, 32 * j),
                        )
                for ci in range(NCH):
                    for j in range(NSUB):
                        sl = slice(SUB * j, SUB * (j + 1))
                        erow = e2_all[32 * j:32 * j + 1, :]
                        nc.tensor.matmul(
                            erow, ones_c, sq_tiles[ci][:, sl],
                            start=(ci == 0), stop=(ci == NCH - 1),
                            tile_position=(0, 32 * j),
                        )
                return b, zt, mu_all, e2_all, s0

             def stage1b(st):
                """stats chain + broadcasts + copies.

                Emitted AFTER the previous macro's tmul block so the DVE
                queue never blocks the value phase on the stats chain.
                """
                b, zt, mu_all, e2_all, s0 = st
                # rstd/mb written BF16 so the K=1 broadcast matmuls run at
                # PE bf16 rate (f32 matmuls are ~4x slower / column).
                musq = rowp.tile([97, SUB], f32, tag="musq")
                nc.scalar.activation(musq, mu_all, AF.Square)
                var = rowp.tile([97, SUB], f32, tag="var")
                nc.vector.scalar_tensor_tensor(
                    var, e2_all, float(EPS), musq, op0=OP.add, op1=OP.subtract,
                )
                rstd = rowp.tile([97, SUB], bf16, tag="rstd")
                # Abs_reciprocal_sqrt = rsqrt(|x|); var >= 0 so identical to
                # rsqrt, and unlike AF.Rsqrt it is not accuracy-gated.
                nc.scalar.activation(
                    rstd, var, AF.Abs_reciprocal_sqrt, scale=inv_k2,
                )
                mb_row = rowp.tile([97, SUB], bf16, tag="mb_row")
                nc.vector.scalar_tensor_tensor(
                    mb_row, mu_all, -1.0, rstd, op0=OP.mult, op1=OP.mult,
                )

                rst16 = bcastp.tile([128, MACRO], bf16, tag="rst16")
                mb16 = bcastp.tile([128, MACRO], bf16, tag="mb16")
                ncopy = 0
                for half in range(2):
                    hsl = slice(1024 * half, 1024 * (half + 1))
                    rp = bcp.tile([128, 1024], f32, name="rp", tag="bc")
                    mp = bcp.tile([128, 1024], f32, name="mp", tag="bc")
                    for jj in range(2):
                        j = 2 * half + jj
                        pr = 32 * j
                        psl = slice(SUB * jj, SUB * (jj + 1))
                        nc.tensor.matmul(
                            rp[:, psl], ones97[pr:pr + 1, :],
                            rstd[pr:pr + 1, :],
                            start=True, stop=True, tile_position=(pr, 0),
                        )
                        nc.tensor.matmul(
                            mp[:, psl], ones97[pr:pr + 1, :],
                            mb_row[pr:pr + 1, :],
                            start=True, stop=True, tile_position=(pr, 0),
                        )
                    for dst, src in ((rst16, rp), (mb16, mp)):
                        if ncopy < CP_DVE:
                            nc.vector.tensor_copy(dst[:, hsl], src)
                        else:
                            nc.scalar.copy(dst[:, hsl], src)
                        ncopy += 1

                return b, zt, rst16, mb16, s0

             def stage2_tmul(vst):
                """value multiplies t = z * rst16"""
                b, zt, rst16, mb16, s0 = vst
                t_tiles = []
                for ci in range(NCH):
                    t16 = t16p.tile([128, MACRO], bf16, tag="t16")
                    nc.vector.tensor_mul(t16, zt[:, ci, :], rst16)
                    t_tiles.append(t16)
                return t_tiles

             def stage2_yadd(vst, t_tiles):
                """value adds y = t + mb16 + output DMA"""
                b, zt, rst16, mb16, s0 = vst
                for ci in range(NCH):
                    yt = yp.tile([128, MACRO], out_dt, name=f"yc{ci}",
                                 tag="yc")
                    nc.vector.tensor_tensor(yt, t_tiles[ci], mb16, op=OP.add)
                    if not trivial_affine:
                        nc.vector.tensor_scalar(
                            yt, yt,
                            g_sb[:, ci:ci + 1], b_sb[:, ci:ci + 1],
                            op0=OP.mult, op1=OP.add,
                        )
                    nc.sync.dma_start(yv[b, :, ci, s0:s0 + MACRO], yt)

             bms = [(b, m) for b in range(BPC) for m in range(NMACRO)]
             pending = None
             for k, bm in enumerate(bms):
                st = stage1a(*bm, first=(k == 0))
                if pending is not None:
                    t_tiles = stage2_tmul(pending)
                vst = stage1b(st)
                if pending is not None:
                    stage2_yadd(pending, t_tiles)
                pending = vst
             t_tiles = stage2_tmul(pending)
             stage2_yadd(pending, t_tiles)

    nc.compile()
    return nc


def _get_module(trivial_affine: bool):
    key = (trivial_affine, GPS_YADD, GPS_SQ, CP_DVE, OUT_I8, MACRO)
    if key not in _CACHE:
        _CACHE[key] = _build(trivial_affine)
    return _CACHE[key]


def kernel(**inputs) -> np.ndarray:
    global LAST_RESULTS
    from concourse.bass_utils import run_bass_kernel_spmd

    x = np.asarray(inputs["x"], dtype=np.float32)
    te = np.asarray(inputs["text_emb"], dtype=np.float32)
    Wv = np.asarray(inputs["Wv"], dtype=np.float32)
    Wo = np.asarray(inputs["Wo"], dtype=np.float32)
    bo = np.asarray(inputs["bo"], dtype=np.float32)
    gamma = np.asarray(inputs["gamma"], dtype=np.float32)
    beta = np.asarray(inputs["beta"], dtype=np.float32)
    assert x.shape == (B, C, H, W), x.shape

    trivial = bool(np.all(gamma == 1.0) and np.all(beta == 0.0))
    nc = _get_module(trivial)

    # host-side tiny matvec chain: p[b] = (te @ Wv.T) @ Wo.T + bo
    p = (te @ Wv.T) @ Wo.T + bo  # (B, C) f32
    pcol = np.ascontiguousarray(
        p.reshape(B, NCH, 128).transpose(2, 1, 0)
    )  # (128, NCH, B)

    x16 = np.ascontiguousarray(
        x.reshape(B, C, S).astype(ml_dtypes.bfloat16)
    )

    in_maps = []
    for c in range(NCORES):
        bsl = slice(BPC * c, BPC * (c + 1))
        m = {
            "x": np.ascontiguousarray(x16[bsl]),
            "pcol": np.ascontiguousarray(pcol[:, :, bsl]),
        }
        if not trivial:
            m["gcols"] = np.ascontiguousarray(gamma.reshape(NCH, 128).T)
            m["bcols"] = np.ascontiguousarray(beta.reshape(NCH, 128).T)
        in_maps.append(m)

    kwargs = {}
    if TRACE:
        import os
        import shutil

        shutil.rmtree("/tmp/bassprof", ignore_errors=True)
        os.makedirs("/tmp/bassprof", exist_ok=True)
        kwargs["tmpdir"] = "/tmp/bassprof"
    res = run_bass_kernel_spmd(
        nc, in_maps, core_ids=list(range(NCORES)), trace=TRACE, **kwargs
    )
    LAST_RESULTS = res
    out = np.concatenate(
        [np.asarray(res.results[c]["y"]) for c in range(NCORES)], axis=0
    )
    out = out.astype(np.float32)
    if OUT_I8:
        out *= I8_SCALE
    return np.ascontiguousarray(out.reshape(B, C, H, W))


# revision 29
# speedup vs baseline: 1.1068x; 1.0038x over previous
"""Trainium2 Bass kernel for nn_CrossAttentionBlock_73452530696666.

Math note: the reference's attention softmax runs over a single KV token, so
attn == 1.0 exactly and the whole q/scores path is dead code. The output
reduces to, per batch b and spatial position s:

    p[b]   = (text_emb[b] @ Wv.T) @ Wo.T + bo          # (C,) per batch
    y[:,s] = LayerNorm_C(x[:, s] + p[b]) * gamma + beta

p is a tiny per-batch matvec chain -> computed on host. The device kernel is
a fused bias-add + LayerNorm over the channel dim streamed over (B, C, H*W).

v4 design (HW-measured op costs per [128,2048] bf16: DVE TS@4x 746ns,
DVE TT@2x 1217ns, DVE STT 1x 2813ns (!), ACT Square 1819ns, ACT
Square+biasAP 2414ns (!), ACT copy [128,1024] 1146ns, PE f32 K=1 matmul
~860-1460ns vs bf16 ~300ns):
  - baseline value path kept (z=TS(x+p)@4x in-place, t=TT(z,rst16),
    y=TT(t,mb16)) -- STT fusion is 1x on HW, a loss.
  - stats rows (rstd, mb) written BF16 so the K=1 partition-broadcast
    matmuls run at bf16 rate on PE (f32 matmuls are ~4x slower).
  - GPS_YADD value-phase adds offloaded to gpsimd (only spare engine).
  - xp bufs=4: the whole per-core x input stays resident -> input DMA
    never stalls on buffer recycling.
  - optional int8 output (scale folded into rstd/mb), host dequant.

Sharding: data-parallel over batch, 2 batches per core on 8 cores.
"""

import sys

sys.path.insert(0, "/opt/trn_rl_repo")

import ml_dtypes
import numpy as np

B, C, H, W, T = 16, 512, 64, 64, 768
S = H * W  # 4096
NCORES = 8
BPC = B // NCORES  # batches per core = 2
NCH = C // 128  # channel chunks = 4
MACRO = 2048  # spatial columns per macro tile
SUB = 512  # matmul / PSUM sub tile
NSUB = MACRO // SUB  # 4
NMACRO = S // MACRO  # 2 per batch
EPS = 1e-5

# Tuning knobs (rebuild to change):
GPS_YADD = 0  # value-phase yadd TTs per macro on gpsimd (0..4) -- gpsimd
              # shares the DVE SBUF port; >0 measured as a net loss
GPS_SQ = 0    # square TTs per macro on gpsimd (0..4)
CP_DVE = 1    # of the 4 [128,1024] PSUM->SBUF bcast copies per macro, on DVE
OUT_I8 = False  # write y as int8: DVE TT with i8 output drops to 1x (+17us
                # DVE, measured) for -4us of DMA -- net loss, keep bf16

I8_SCALE = 5.0 / 127.0  # y ~ N(0,1) per token; clip at 5 sigma

# Set by test harness to request a profiled run.
TRACE = False
LAST_RESULTS = None

_CACHE = {}


def _build(trivial_affine: bool):
    import concourse.bass as bass
    import concourse.tile as tile
    from concourse import bacc, mybir

    f32 = mybir.dt.float32
    bf16 = mybir.dt.bfloat16
    i8 = mybir.dt.int8
    AF = mybir.ActivationFunctionType
    OP = mybir.AluOpType

    out_dt = i8 if OUT_I8 else bf16
    # int8 scale k folded into rstd (and thus mb = -mu*rstd) via the
    # rsqrt activation's scale: rsqrt(var/k^2) = k*rstd.
    k = 1.0 / I8_SCALE if OUT_I8 else 1.0
    inv_k2 = 1.0 / (k * k)

    nc = bacc.Bacc("TRN2", target_bir_lowering=False)
    x = nc.dram_tensor("x", (BPC, C, S), bf16, kind="ExternalInput")
    pcol = nc.dram_tensor("pcol", (128, NCH, BPC), f32, kind="ExternalInput")
    if not trivial_affine:
        gcols = nc.dram_tensor("gcols", (128, NCH), f32, kind="ExternalInput")
        bcols = nc.dram_tensor("bcols", (128, NCH), f32, kind="ExternalInput")
    y = nc.dram_tensor("y", (BPC, C, S), out_dt, kind="ExternalOutput")

    xv = x.rearrange("b (n p) s -> b p n s", p=128)
    yv = y.rearrange("b (n p) s -> b p n s", p=128)

    with tile.TileContext(nc) as tc:
        with tc.tile_pool(name="consts", bufs=1) as consts:
            ones_c = consts.tile([128, 1], bf16)
            nc.vector.memset(ones_c, 1.0 / C)  # lhsT for channel-mean matmuls
            ones97 = consts.tile([97, 128], bf16)
            nc.vector.memset(ones97, 1.0)  # bf16 lhsT rows for K=1 broadcasts
            pcol_sb = consts.tile([128, NCH, BPC], f32)
            nc.sync.dma_start(pcol_sb, pcol[:, :, :])
            # preload both ACT function tables during the first x DMA
            warm = consts.tile([1, 1], f32)
            nc.scalar.activation(warm, ones_c[0:1, 0:1], AF.Square)
            warm2 = consts.tile([1, 1], f32)
            nc.scalar.activation(warm2, warm, AF.Abs_reciprocal_sqrt)
            if not trivial_affine:
                g_sb = consts.tile([128, NCH], f32)
                nc.sync.dma_start(g_sb, gcols[:, :])
                b_sb = consts.tile([128, NCH], f32)
                nc.sync.dma_start(b_sb, bcols[:, :])

            with (
                tc.tile_pool(name="xp", bufs=4) as xp,
                tc.tile_pool(name="sqp", bufs=6) as sqp,
                tc.tile_pool(name="bcastp", bufs=4) as bcastp,
                tc.tile_pool(name="rowp", bufs=6) as rowp,
                tc.tile_pool(name="t16p", bufs=4) as t16p,
                tc.tile_pool(name="yp", bufs=6) as yp,
                tc.tile_pool(name="mup", bufs=2, space="PSUM") as mup,
                tc.tile_pool(name="e2p", bufs=2, space="PSUM") as e2p,
                tc.tile_pool(name="bcp", bufs=2, space="PSUM") as bcp,
            ):
             def stage1a(b, m, first=False):
                """load + z + sq + PE reductions (no stats chain)

                first=True (pipeline-fill phase, input-DMA starved): the
                DMA, z and sq are emitted at finer granularity so compute
                starts on the first quarter instead of the full 2MB; the
                small-tile per-op overhead is free while DVE/ACT idle.
                """
                s0 = m * MACRO
                xt = xp.tile([128, NCH, MACRO], bf16, name="xt")
                # input DMA always quartered: SP issue is free, SDMA chunks
                # stay >=512KB, and region tracking lets consumers start early
                q = MACRO // 4
                for qi in range(4):
                    nc.sync.dma_start(
                        xt[:, :, q * qi:q * (qi + 1)],
                        xv[b, :, :, s0 + q * qi:s0 + q * (qi + 1)],
                    )

                # z = x + p (bf16, TS @4x), in place over xt
                zt = xt
                nz = 4 if first else 1
                for ci in range(NCH):
                    for qi in range(nz):
                        w = MACRO // nz
                        nc.vector.tensor_scalar_add(
                            zt[:, ci, w * qi:w * (qi + 1)],
                            xt[:, ci, w * qi:w * (qi + 1)],
                            pcol_sb[:, ci, b:b + 1],
                        )

                # sq = z^2 (bf16) per chunk on ACT (halved in fill phase)
                nsq = 2 if first else 1
                sq_tiles = []
                for ci in range(NCH):
                    sq = sqp.tile([128, MACRO], bf16, name=f"sq{ci}", tag="sq")
                    for qi in range(nsq):
                        w = MACRO // nsq
                        nc.scalar.activation(
                            sq[:, w * qi:w * (qi + 1)],
                            zt[:, ci, w * qi:w * (qi + 1)], AF.Square,
                        )
                    sq_tiles.append(sq)

                # channel sums: row j lives at partition 32*j of one bank
                mu_all = mup.tile([97, SUB], f32, name="mu_all")
                e2_all = e2p.tile([97, SUB], f32, name="e2_all")
                for ci in range(NCH):
                    for j in range(NSUB):
                        sl = slice(SUB * j, SUB * (j + 1))
                        mrow = mu_all[32 * j:32 * j + 1, :]
                        nc.tensor.matmul(
                            mrow, ones_c, zt[:, ci, sl],
                            start=(ci == 0), stop=(ci == NCH - 1),
                            tile_position=(0, 32 * j),
                        )
                for ci in range(NCH):
                    for j in range(NSUB):
                        sl = slice(SUB * j, SUB * (j + 1))
                        erow = e2_all[32 * j:32 * j + 1, :]
                        nc.tensor.matmul(
                            erow, ones_c, sq_tiles[ci][:, sl],
                            start=(ci == 0), stop=(ci == NCH - 1),
                            tile_position=(0, 32 * j),
                        )
                return b, zt, mu_all, e2_all, s0

             def stage1b(st):
                """stats chain + broadcasts + copies.

                Emitted AFTER the previous macro's tmul block so the DVE
                queue never blocks the value phase on the stats chain.
                """
                b, zt, mu_all, e2_all, s0 = st
                # rstd/mb written BF16 so the K=1 broadcast matmuls run at
                # PE bf16 rate (f32 matmuls are ~4x slower / column).
                musq = rowp.tile([97, SUB], f32, tag="musq")
                nc.scalar.activation(musq, mu_all, AF.Square)
                var = rowp.tile([97, SUB], f32, tag="var")
                nc.vector.scalar_tensor_tensor(
                    var, e2_all, float(EPS), musq, op0=OP.add, op1=OP.subtract,
                )
                rstd = rowp.tile([97, SUB], bf16, tag="rstd")
                # Abs_reciprocal_sqrt = rsqrt(|x|); var >= 0 so identical to
                # rsqrt, and unlike AF.Rsqrt it is not accuracy-gated.
                nc.scalar.activation(
                    rstd, var, AF.Abs_reciprocal_sqrt, scale=inv_k2,
                )
                mb_row = rowp.tile([97, SUB], bf16, tag="mb_row")
                nc.vector.scalar_tensor_tensor(
                    mb_row, mu_all, -1.0, rstd, op0=OP.mult, op1=OP.mult,
                )

                rst16 = bcastp.tile([128, MACRO], bf16, tag="rst16")
                mb16 = bcastp.tile([128, MACRO], bf16, tag="mb16")
                ncopy = 0
                for half in range(2):
                    hsl = slice(1024 * half, 1024 * (half + 1))
                    rp = bcp.tile([128, 1024], f32, name="rp", tag="bc")
                    mp = bcp.tile([128, 1024], f32, name="mp", tag="bc")
                    for jj in range(2):
                        j = 2 * half + jj
                        pr = 32 * j
                        psl = slice(SUB * jj, SUB * (jj + 1))
                        nc.tensor.matmul(
                            rp[:, psl], ones97[pr:pr + 1, :],
                            rstd[pr:pr + 1, :],
                            start=True, stop=True, tile_position=(pr, 0),
                        )
                        nc.tensor.matmul(
                            mp[:, psl], ones97[pr:pr + 1, :],
                            mb_row[pr:pr + 1, :],
                            start=True, stop=True, tile_position=(pr, 0),
                        )
                    for dst, src in ((rst16, rp), (mb16, mp)):
                        if ncopy < CP_DVE:
                            nc.vector.tensor_copy(dst[:, hsl], src)
                        else:
                            nc.scalar.copy(dst[:, hsl], src)
                        ncopy += 1

                return b, zt, rst16, mb16, s0

             def stage2_tmul(vst):
                """value multiplies t = z * rst16"""
                b, zt, rst16, mb16, s0 = vst
                t_tiles = []
                for ci in range(NCH):
                    t16 = t16p.tile([128, MACRO], bf16, tag="t16")
                    nc.vector.tensor_mul(t16, zt[:, ci, :], rst16)
                    t_tiles.append(t16)
                return t_tiles

             def stage2_yadd(vst, t_tiles):
                """value adds y = t + mb16 + output DMA"""
                b, zt, rst16, mb16, s0 = vst
                for ci in range(NCH):
                    yt = yp.tile([128, MACRO], out_dt, name=f"yc{ci}",
                                 tag="yc")
                    nc.vector.tensor_tensor(yt, t_tiles[ci], mb16, op=OP.add)
                    if not trivial_affine:
                        nc.vector.tensor_scalar(
                            yt, yt,
                            g_sb[:, ci:ci + 1], b_sb[:, ci:ci + 1],
                            op0=OP.mult, op1=OP.add,
                        )
                    nc.sync.dma_start(yv[b, :, ci, s0:s0 + MACRO], yt)

             bms = [(b, m) for b in range(BPC) for m in range(NMACRO)]
             pending = None
             for k, bm in enumerate(bms):
                st = stage1a(*bm, first=(k < 2))
                if pending is not None:
                    t_tiles = stage2_tmul(pending)
                vst = stage1b(st)
                if pending is not None:
                    stage2_yadd(pending, t_tiles)
                pending = vst
             t_tiles = stage2_tmul(pending)
             stage2_yadd(pending, t_tiles)

    nc.compile()
    return nc


def _get_module(trivial_affine: bool):
    key = (trivial_affine, GPS_YADD, GPS_SQ, CP_DVE, OUT_I8, MACRO)
    if key not in _CACHE:
        _CACHE[key] = _build(trivial_affine)
    return _CACHE[key]


def kernel(**inputs) -> np.ndarray:
    global LAST_RESULTS
    from concourse.bass_utils import run_bass_kernel_spmd

    x = np.asarray(inputs["x"], dtype=np.float32)
    te = np.asarray(inputs["text_emb"], dtype=np.float32)
    Wv = np.asarray(inputs["Wv"], dtype=np.float32)
    Wo = np.asarray(inputs["Wo"], dtype=np.float32)
    bo = np.asarray(inputs["bo"], dtype=np.float32)
    gamma = np.asarray(inputs["gamma"], dtype=np.float32)
    beta = np.asarray(inputs["beta"], dtype=np.float32)
    assert x.shape == (B, C, H, W), x.shape

    trivial = bool(np.all(gamma == 1.0) and np.all(beta == 0.0))
    nc = _get_module(trivial)

    # host-side tiny matvec chain: p[b] = (te @ Wv.T) @ Wo.T + bo
    p = (te @ Wv.T) @ Wo.T + bo  # (B, C) f32
    pcol = np.ascontiguousarray(
        p.reshape(B, NCH, 128).transpose(2, 1, 0)
    )  # (128, NCH, B)

    x16 = np.ascontiguousarray(
        x.reshape(B, C, S).astype(ml_dtypes.bfloat16)
    )

    in_maps = []
    for c in range(NCORES):
        bsl = slice(BPC * c, BPC * (c + 1))
        m = {
            "x": np.ascontiguousarray(x16[bsl]),
            "pcol": np.ascontiguousarray(pcol[:, :, bsl]),
        }
        if not trivial:
            m["gcols"] = np.ascontiguousarray(gamma.reshape(NCH, 128).T)
            m["bcols"] = np.ascontiguousarray(beta.reshape(NCH, 128).T)
        in_maps.append(m)

    kwargs = {}
    if TRACE:
        import os
        import shutil

        shutil.rmtree("/tmp/bassprof", ignore_errors=True)
        os.makedirs("/tmp/bassprof", exist_ok=True)
        kwargs["tmpdir"] = "/tmp/bassprof"
    res = run_bass_kernel_spmd(
        nc, in_maps, core_ids=list(range(NCORES)), trace=TRACE, **kwargs
    )
    LAST_RESULTS = res
    out = np.concatenate(
        [np.asarray(res.results[c]["y"]) for c in range(NCORES)], axis=0
    )
    out = out.astype(np.float32)
    if OUT_I8:
        out *= I8_SCALE
    return np.ascontiguousarray(out.reshape(B, C, H, W))
